# revision 23
# baseline (speedup 1.0000x reference)
"""GraphSage 2-level mean-aggregator GNN on 8 trn2 NeuronCores.

Strategy (memory-bound problem: dif_mat_1 is [6000, 48000] f32 = 1.15 GB and
must stream through the chip exactly once):

  * Shard the level-1 diffusion matmul over its CONTRACTION dim (the 48000
    src selections): core k owns src columns [6000k, 6000(k+1)).  The dif
    shard AND the stationary src features are quantized host-side to
    fp8-e4m3 (dif scaled by 2^23; the product is descaled on the PSUM
    copy-out), packed partition-major so every stream DMA is a contiguous
    [128, x] slab with ~12 KB per-partition lines — ~37 MB/core of traffic.
  * The stream matmuls run in DoubleRow perf mode (2 contraction k-tiles
    per instruction, 2x PE throughput) producing a partial
    agg1T = src_feat^T @ dif^T of shape [128, 6000].  Slab DMAs alternate
    between the sync and scalar HWDGE rings; partial copies out to DRAM
    ride the same rings two slabs late (so their data is always ready and
    they never stall the stream), keeping every transfer at full HWDGE
    rate — SWDGE traffic gets starved while the stream saturates HBM.
  * Partials are copied out as fp8 (scaled by 2^13, descaled via a
    host-side 2^-13 on the w1 agg-half) and AllReduce-summed in 3 slices
    (1024 / 2560 / 2416 dst cols).  Slice 0 finishes during the stream;
    slices 1/2 run back-to-back right after it (the cc engine is near
    useless mid-stream, ~3-8 GB/s vs ~25 GB/s clean, and costs ~15-25 us
    per op regardless of size, so op count is kept minimal).  The
    dense-phase constants (d0e, dfT) load at the tail of the stream rings
    so the stream itself stays ~9 us shorter.
  * Every core then runs the small level-1 dense layer (relu(concat @ w1))
    redundantly, tiled so h1 chunks feed straight into the level-0
    aggregation as stationary matmul operands.
  * Level 0 is sharded by the 1024 target nodes (128 rows/core).  The
    h1[src_idx_0] gather AND the h1[dst_idx_0] selection are folded into one
    host-built matrix [difT0exp | one-hot-E] of shape [6016, 256] (fp8-e4m3
    scaled by 2^7, descaled on the c0 copy), so the whole level collapses
    into one accumulated matmul over the h1 chunks.
  * Final dense + softmax per core on its 128 rows; host concatenates the
    eight [128, 40] outputs.  No device-side transposes or gathers needed.

All activations flow through the TensorEngine in natural layouts:
lhsT = [K, M] / rhs = [K, N] with the contraction dim always on partitions.
"""

import sys

import ml_dtypes
import numpy as np

sys.path.insert(0, "/opt/trn_rl_repo")

from concourse import bacc, bass_utils, mybir, tile

F32 = mybir.dt.float32
BF16 = mybir.dt.bfloat16
FP8E4 = mybir.dt.float8e4

# Problem dims (hardcoded per spec)
N, F = 100000, 128
N1, D1, S1 = 60000, 6000, 48000
D0, S0 = 1024, 5000
H, C = 128, 40
NCORES = 8
CH = S1 // NCORES  # 6000 src columns per core
KT = 47            # contraction k-tiles of 128 (6016 = padded 6000)
KP = KT * 128      # 6016
JW = 512           # dst-block width for the big matmul
JB = 12            # dst blocks; the last is 368 wide (11*512 + 368 = 6000)
JWL = D1 - (JB - 1) * JW  # 368
D0SH = D0 // NCORES  # 128 target rows per core
SCALE = np.float32(2.0 ** 23)   # fp8 dif scale; descaled on PSUM copy-out
ISCALE = float(1.0 / SCALE)     # multiplied by PSH on the partial copy-out
PSH = np.float32(2.0 ** 13)     # fp8 partial scale; descaled via w1a host-side
SC0 = np.float32(2.0 ** 7)      # fp8 d0e scale; descaled on c0 copy
ISC0 = float(1.0 / SC0)
HKT = 24           # k-tiles in the first half of a stream slab
# AllReduce groups: (first j, last j exclusive, dst-col offset, width)
ARG = [(0, 2, 0, 1024), (2, 9, 1024, 3584), (9, 12, 4608, 1392)]
ARW_LAG = 2        # slabs between a partial's matmuls and its DRAM copy


def _grp(j):
    return next(i for i, a in enumerate(ARG) if a[0] <= j < a[1])


TRACE = False
TRACE_KW = {}
LAST = None  # last BassKernelResults (exec_time_ns when TRACE)

_nc = None


def _build(repeat=1):
    nc = bacc.Bacc(
        "TRN2",
        target_bir_lowering=False,
        debug=False,
        enable_asserts=False,
        num_devices=NCORES,
    )
    difp = nc.dram_tensor(
        "difp", [JB - 1, 128, KT, JW], FP8E4, kind="ExternalInput"
    )
    difl = nc.dram_tensor("difl", [128, KT, JWL], FP8E4, kind="ExternalInput")
    sfp = nc.dram_tensor("sfp", [128, KT, F], FP8E4, kind="ExternalInput")
    dfT = nc.dram_tensor("dfT", [F, KP], BF16, kind="ExternalInput")
    d0ep = nc.dram_tensor("d0ep", [128, KT, 2 * H], FP8E4, kind="ExternalInput")
    w1t = nc.dram_tensor("w1t", [2 * F, H], BF16, kind="ExternalInput")
    w2t = nc.dram_tensor("w2t", [2 * H, H], BF16, kind="ExternalInput")
    wct = nc.dram_tensor("wct", [H, C], BF16, kind="ExternalInput")
    outd = nc.dram_tensor("out", [D0SH, C], F32, kind="ExternalOutput")

    rg = [list(range(NCORES))]
    relu = mybir.ActivationFunctionType.Relu
    DR = mybir.MatmulPerfMode.DoubleRow

    with tile.TileContext(nc) as tc:
        with (
            tc.tile_pool(name="const", bufs=1) as constp,
            tc.tile_pool(name="stream", bufs=6) as streamp,
            tc.tile_pool(name="stage", bufs=4) as stagep,
            tc.tile_pool(name="h1p", bufs=3) as h1p,
            tc.tile_pool(name="ps1p", bufs=2, space="PSUM") as ps1p,
            tc.tile_pool(name="ps0p", bufs=1, space="PSUM") as ps0p,
            tc.tile_pool(name="ps2p", bufs=2, space="PSUM") as ps2p,
            tc.tile_pool(name="ps34p", bufs=1, space="PSUM") as ps34p,
            tc.tile_pool(name="dram", bufs=1, space="DRAM") as dramp,
        ):
            S_sb = constp.tile([128, KT, F], FP8E4, name="S_sb")
            dfT_sb = constp.tile([F, KP], BF16, name="dfT_sb")
            d0e_sb = constp.tile([128, KT, 2 * H], FP8E4, name="d0e_sb")
            w1_sb = constp.tile([128, 2, H], BF16, name="w1_sb")
            w2_sb = constp.tile([128, 2, H], BF16, name="w2_sb")
            wc_sb = constp.tile([H, C], BF16, name="wc_sb")
            ars_sb = constp.tile([F, KP], FP8E4, name="ars_sb")

            # stream-critical constant first on the sync ring; the rest on
            # the gpsimd (SWDGE) ring — those transfers trickle in the
            # background behind the stream and are only needed ~130 us in
            nc.sync.dma_start(S_sb[:], sfp.ap())
            nc.gpsimd.dma_start(
                w1_sb[:], w1t.ap().rearrange("(c p) e -> p c e", p=128)
            )
            nc.gpsimd.dma_start(
                w2_sb[:], w2t.ap().rearrange("(c p) e -> p c e", p=128)
            )
            nc.gpsimd.dma_start(wc_sb[:], wct.ap())
            # the dense loop reads h1 rows 6000..6016 whose agg columns are
            # never streamed; zero them so no NaN garbage flows through relu
            # (their d0e rows are zero, but NaN * 0 = NaN)
            nc.vector.memset(ars_sb[:, D1:KP], 0.0)

            # ---- big streamed matmul: agg1T partial [128, 6000] ----
            # (`repeat` re-runs the whole pipeline for benchmark slope timing;
            # results are identical each rep so output is unchanged)
            for _rep in range(repeat):
              ar_ins, ar_outs = [], []
              for g, (_, _, goff, gw) in enumerate(ARG):
                ai = dramp.tile([F, gw], FP8E4, name=f"ar_in{_rep}_{g}")
                ao = dramp.tile(
                    [F, gw], FP8E4, name=f"ar_out{_rep}_{g}", addr_space="Shared"
                )
                ar_ins.append(ai)
                ar_outs.append(ao)

              pend = {}

              def _flush(j):
                  g = _grp(j)
                  w, st = pend.pop(j)
                  off = j * JW - ARG[g][2]
                  eng = nc.sync if j % 2 == 0 else nc.scalar
                  eng.dma_start(ar_ins[g][:, off : off + w], st[:, 0:w])
                  if j == ARG[g][1] - 1:
                      nc.gpsimd.collective_compute(
                          "AllReduce",
                          mybir.AluOpType.add,
                          replica_groups=rg,
                          ins=[ar_ins[g].opt()],
                          outs=[ar_outs[g].opt()],
                      )

              for j in range(JB):
                # copy out the partial from ARW_LAG slabs ago — its data is
                # long ready, so the write never stalls the stream rings
                if j - ARW_LAG in pend:
                    _flush(j - ARW_LAG)
                w = JW if j < JB - 1 else JWL
                # two half-slab DMAs per dst block on the two HWDGE rings so
                # the first matmul only waits on half the 3 MB transfer
                ra = streamp.tile([128, HKT, JW], FP8E4, tag="ra")
                rb = streamp.tile([128, KT - HKT, JW], FP8E4, tag="rb")
                if j < JB - 1:
                    nc.sync.dma_start(ra[:], difp.ap()[j, :, 0:HKT, :])
                    nc.scalar.dma_start(rb[:], difp.ap()[j, :, HKT:KT, :])
                else:
                    nc.sync.dma_start(ra[:, :, 0:JWL], difl.ap()[:, 0:HKT, :])
                    nc.scalar.dma_start(
                        rb[:, :, 0:JWL], difl.ap()[:, HKT:KT, :]
                    )
                ps1 = ps1p.tile([F, JW], F32, tag="ps1")
                for p in range(HKT // 2):
                    nc.tensor.matmul(
                        ps1[:, 0:w],
                        S_sb[:, 2 * p : 2 * p + 2, :],
                        ra[:, 2 * p : 2 * p + 2, 0:w],
                        start=(p == 0),
                        stop=False,
                        perf_mode=DR,
                    )
                for p in range(HKT // 2, KT // 2):
                    q = 2 * p - HKT
                    nc.tensor.matmul(
                        ps1[:, 0:w],
                        S_sb[:, 2 * p : 2 * p + 2, :],
                        rb[:, q : q + 2, 0:w],
                        start=False,
                        stop=False,
                        perf_mode=DR,
                    )
                nc.tensor.matmul(
                    ps1[:, 0:w],
                    S_sb[:, KT - 1, :],
                    rb[:, KT - 1 - HKT, 0:w],
                    start=False,
                    stop=True,
                )
                st = stagep.tile([F, JW], FP8E4, tag="st")
                nc.vector.tensor_scalar_mul(
                    st[:, 0:w], ps1[:, 0:w], float(ISCALE * PSH)
                )
                pend[j] = (w, st)
                if ARW_LAG == 0:
                    _flush(j)
              for j in sorted(pend):
                  _flush(j)

              # dense-phase constants land right as the stream drains
              nc.sync.dma_start(d0e_sb[:], d0ep.ap())
              nc.scalar.dma_start(dfT_sb[:], dfT.ap())
              # summed slices come back on the sync ring, after the stream
              for g in (0, 1, 2):
                goff, gw = ARG[g][2], ARG[g][3]
                nc.sync.dma_start(
                    ars_sb[:, goff : goff + gw], ar_outs[g][:]
                )

              # ---- level-1 dense + level-0 aggregation, fused per h1 chunk ----
              # the dst-feature half of the dense doesn't depend on the
              # AllReduce — precompute it while the collective is in flight
              # (this also keeps the PE busy so HAM doesn't re-throttle)
              zdst_sb = constp.tile([128, KT, H], BF16, name="zdst_sb")
              for t in range(KT):
                psz = ps2p.tile([128, H], F32, tag="ps2")
                nc.tensor.matmul(
                    psz[:],
                    dfT_sb[:, t * 128 : (t + 1) * 128],
                    w1_sb[:, 1, :],
                    start=True,
                    stop=True,
                )
                nc.scalar.copy(zdst_sb[:, t, :], psz[:])

              ps0 = ps0p.tile([H, 2 * H], F32, name="ps0")
              for ti, t in enumerate(range(KT)):
                ps2 = ps2p.tile([128, H], F32, tag="ps2")
                nc.tensor.matmul(
                    ps2[:],
                    ars_sb[:, t * 128 : (t + 1) * 128],
                    w1_sb[:, 0, :],
                    start=True,
                    stop=True,
                )
                h1s = h1p.tile([128, H], F32, tag="h1s")
                nc.vector.tensor_tensor(
                    h1s[:], ps2[:], zdst_sb[:, t, :], mybir.AluOpType.add
                )
                h1t = h1p.tile([128, H], BF16, tag="h1t")
                nc.scalar.activation(h1t[:], h1s[:], relu)
                nc.tensor.matmul(
                    ps0[:],
                    h1t[:],
                    d0e_sb[:, t, :],
                    start=(ti == 0),
                    stop=(ti == KT - 1),
                )

            # ---- level-0 dense + classifier + softmax ----
            c0_sb = constp.tile([H, 2 * H], BF16, name="c0_sb")
            nc.vector.tensor_scalar_mul(c0_sb[:], ps0[:], ISC0)
            ps3 = ps34p.tile([H, D0SH], F32, tag="ps34")
            nc.tensor.matmul(
                ps3[:], w2_sb[:, 0, :], c0_sb[:, 0:H], start=True, stop=False
            )
            nc.tensor.matmul(
                ps3[:], w2_sb[:, 1, :], c0_sb[:, H : 2 * H], start=False, stop=True
            )
            h0T = constp.tile([H, D0SH], BF16, name="h0T")
            nc.scalar.activation(h0T[:], ps3[:], relu)
            ps4 = ps34p.tile([D0SH, C], F32, tag="ps34")
            nc.tensor.matmul(ps4[:], h0T[:], wc_sb[:], start=True, stop=True)

            mx = constp.tile([D0SH, 1], F32, name="mx")
            nc.vector.tensor_reduce(
                mx[:], ps4[:], axis=mybir.AxisListType.X, op=mybir.AluOpType.max
            )
            nmx = constp.tile([D0SH, 1], F32, name="nmx")
            nc.vector.tensor_scalar_mul(nmx[:], mx[:], -1.0)
            esb = constp.tile([D0SH, C], F32, name="esb")
            ssum = constp.tile([D0SH, 1], F32, name="ssum")
            nc.scalar.activation(
                esb[:],
                ps4[:],
                mybir.ActivationFunctionType.Exp,
                bias=nmx[:],
                accum_out=ssum[:],
            )
            rs = constp.tile([D0SH, 1], F32, name="rs")
            nc.vector.reciprocal(rs[:], ssum[:])
            osb = constp.tile([D0SH, C], F32, name="osb")
            nc.vector.tensor_scalar_mul(osb[:], esb[:], rs[:])
            nc.sync.dma_start(outd.ap(), osb[:])

    nc.compile()
    return nc


def _prep_in_maps(
    features,
    src_nodes,
    dst_idx_1,
    src_idx_1,
    dif_mat_1,
    dst_idx_0,
    src_idx_0,
    dif_mat_0,
    w1,
    w2,
    w_cls,
):
    f32 = np.float32
    bf16 = ml_dtypes.bfloat16
    fp8 = ml_dtypes.float8_e4m3
    features = np.asarray(features, f32)
    dif_mat_1 = np.asarray(dif_mat_1, f32)
    dif_mat_0 = np.asarray(dif_mat_0, f32)
    src_nodes = np.asarray(src_nodes)
    gsrc = src_nodes[np.asarray(src_idx_1)]  # [48000] rows into features
    gdst = src_nodes[np.asarray(dst_idx_1)]  # [6000]

    dfT = np.zeros((F, KP), f32)
    dfT[:, :D1] = features[gdst].T

    difT0exp = np.zeros((KP, D0), f32)
    np.add.at(difT0exp, np.asarray(src_idx_0), dif_mat_0.T)
    E = np.zeros((KP, D0), f32)
    E[np.asarray(dst_idx_0), np.arange(D0)] = 1.0

    w1s = np.array(w1, f32, copy=True)
    w1s[:F] *= float(1.0 / PSH)   # descale the fp8 agg partials
    w1c = np.ascontiguousarray(w1s).astype(bf16)
    w2c = np.ascontiguousarray(w2).astype(bf16)
    wcc = np.ascontiguousarray(w_cls).astype(bf16)
    dfT16 = dfT.astype(bf16)

    in_maps = []
    for k in range(NCORES):
        sl = slice(k * CH, (k + 1) * CH)
        # fp8 stream, packed [JB-1, 128, KT, JW] + narrow last block:
        #   difp[j, p, kt, e] = dif[src = kt*128+p, dst = j*512+e] * SCALE
        P = np.zeros((KP, D1), f32)
        P[:CH, :] = dif_mat_1[:, sl].T
        Q = (P * SCALE).astype(fp8)                      # [KP, 6000]
        full = Q[:, : (JB - 1) * JW].reshape(KT, 128, JB - 1, JW)
        difp = np.ascontiguousarray(full.transpose(2, 1, 0, 3))
        difl = np.ascontiguousarray(
            Q[:, (JB - 1) * JW :].reshape(KT, 128, JWL).transpose(1, 0, 2)
        )

        # src features (unscaled fp8), packed [128, KT, F]
        sf = np.zeros((KP, F), f32)
        sf[:CH] = features[gsrc[sl]]
        sfp = np.ascontiguousarray(
            sf.astype(fp8).reshape(KT, 128, F).transpose(1, 0, 2)
        )

        # [difT0exp | E] columns for this core's targets, packed [128, KT, 2H]
        d0e = np.zeros((KP, 2 * H), f32)
        d0e[:, :H] = difT0exp[:, k * D0SH : (k + 1) * D0SH]
        d0e[:, H:] = E[:, k * D0SH : (k + 1) * D0SH]
        d0ep = np.ascontiguousarray(
            (d0e * SC0).astype(fp8).reshape(KT, 128, 2 * H).transpose(1, 0, 2)
        )

        in_maps.append(
            {
                "difp": difp,
                "difl": difl,
                "sfp": sfp,
                "dfT": dfT16,
                "d0ep": d0ep,
                "w1t": w1c,
                "w2t": w2c,
                "wct": wcc,
            }
        )
    return in_maps


def kernel(**inputs):
    global _nc, LAST
    if _nc is None:
        _nc = _build()
    in_maps = _prep_in_maps(**inputs)
    res = bass_utils.run_bass_kernel_spmd(
        _nc,
        in_maps,
        core_ids=list(range(NCORES)),
        trace=TRACE,
        **TRACE_KW,
    )
    LAST = res
    out = np.concatenate([res.results[k]["out"] for k in range(NCORES)], axis=0)
    return out.astype(np.float32)
